# revision 20
# baseline (speedup 1.0000x reference)
"""Multi-head attention (B=2, N=2048, C=1024, H=16, D=64) on 8 Trainium2
NeuronCores.

Sharding: tensor-parallel over heads x data-parallel over batch.
Core (b, g) with b in {0,1}, g in {0..3} handles batch b and heads
[4g, 4g+4). Each core computes qkv for its heads, attention, and a partial
output projection (row-parallel); the host sums the 4 partials per batch and
adds the bias.

Per-core kernel layout (all matmuls in fp32r — full fp32 storage, reduced
multiplier precision, 1 PE cycle/row):
  qT/kT [d, n] via lhsT=w^T, rhs=x^T          (d on partitions, pair-packed)
  scoresT[j, i] = kT.T @ qT                   (two K=64 row-tiled matmuls)
  attnT = exp(scale * scoresT)                (ACT, PSUM->SBUF, no max pass)
  aoT[d, i] += [v | 1]^T @ attnT              (row 64 = softmax denominators)
  aoT *= 1/sums (broadcast), then out = aoT.T @ wpT partial projection.
"""
import numpy as np
import sys

sys.path.insert(0, "/opt/trn_rl_repo")

B = 2
N = 2048
C = 1024
H = 16
D = 64
SCALE = D ** -0.5

HEADS_PER_CORE = 4  # 2 pairs
N_CORES = 8

_cache = {}


def _build():
    import concourse.bass as bass
    import concourse.tile as tile
    from concourse import bacc, mybir

    F32 = mybir.dt.float32
    F32R = mybir.dt.float32r
    P = 128
    NC4 = N // 512   # 4 i-chunks of 512
    NB = N // P      # 16 n/j blocks of 128
    CO = C // P      # 8 contraction subtiles

    nc = bacc.Bacc("TRN2", target_bir_lowering=False, debug=False)
    xT = nc.dram_tensor("xT", (C, N), F32, kind="ExternalInput")
    wqkT = nc.dram_tensor("wqkT", (C, 512), F32, kind="ExternalInput")
    wvT = nc.dram_tensor("wvT", (C, 256), F32, kind="ExternalInput")
    wpT = nc.dram_tensor("wpT", (256, C), F32, kind="ExternalInput")
    out = nc.dram_tensor("out", (N, C), F32, kind="ExternalOutput")

    with tile.TileContext(nc) as tc:
        with (
            tc.tile_pool(name="big", bufs=1) as big,
            tc.tile_pool(name="attn", bufs=3) as attn_pool,
            tc.tile_pool(name="norm", bufs=2) as norm_pool,
            tc.tile_pool(name="outp", bufs=3) as out_pool,
            tc.tile_pool(name="ps_mm", bufs=2, space="PSUM") as ps_mm,
            tc.tile_pool(name="ps_sc", bufs=2, space="PSUM") as ps_sc,
            tc.tile_pool(name="ps_av", bufs=1, space="PSUM") as ps_av,
        ):
            # ---- weights + x loads ----
            wqk_sb = big.tile([P, CO, 512], F32R)
            for co in range(CO):
                nc.sync.dma_start(
                    wqk_sb[:, co, :],
                    wqkT.ap()[co * P:(co + 1) * P, :].bitcast(F32R),
                )
            wv_sb = big.tile([P, CO, 256], F32R)
            for co in range(CO):
                nc.sync.dma_start(
                    wv_sb[:, co, :],
                    wvT.ap()[co * P:(co + 1) * P, :].bitcast(F32R),
                )
            wp_sb = big.tile([P, 2, C], F32R)
            for cs in range(2):
                nc.sync.dma_start(
                    wp_sb[:, cs, :],
                    wpT.ap()[cs * P:(cs + 1) * P, :].bitcast(F32R),
                )
            ones_c = big.tile([P, 1], F32)
            nc.vector.memset(ones_c[:], 1.0)
            # Preload the exp ACT table now (~2.7us) so the first real exp in
            # the attention phase doesn't stall the PE past the HAM window.
            exp_warm = big.tile([P, 1], F32)
            nc.scalar.activation(
                out=exp_warm[:], in_=ones_c[:],
                func=mybir.ActivationFunctionType.Exp,
            )

            xT_sb = big.tile([P, CO, N], F32R)
            for co in range(CO):
                nc.sync.dma_start(
                    xT_sb[:, co, :],
                    xT.ap()[co * P:(co + 1) * P, :].bitcast(F32R),
                )

            # PE warm-up: ~8us of junk matmuls on a zeroed tile while input
            # DMAs stream, so the HAM clock-gate is at 8/8 when real work
            # starts. Results go to a scratch psum that is never read.
            import os as _os
            _warmup = _os.environ.get("K_NO_WARMUP") != "1"
            NUM_FILLER = int(_os.environ.get("K_FILLER", "4"))
            warm = big.tile([P, 512], F32R)
            nc.vector.memset(warm[:].bitcast(F32), 0.0)
            wsink = big.tile([P, 512], F32)
            for wu in range(36 if _warmup else 0):
                pw = ps_mm.tile([P, 512], F32, name="pwarm", tag="pm")
                nc.tensor.matmul(
                    pw[:], warm[:, 0:128], warm[:], start=True, stop=True
                )
                if wu % 18 == 17:
                    nc.vector.tensor_copy(wsink[:], pw[:])

            # ---- qT / kT (pair-packed [d(2x64), n]) ----
            # wqkT cols: [q_p0 | k_p0 | q_p1 | k_p1] each 128 wide
            qk_sb = [big.tile([P, N], F32R, name=f"qk_sb{i}") for i in range(4)]
            for fc in range(4):
                for ick in range(NC4):
                    pm = ps_mm.tile([P, 512], F32)
                    for co in range(CO):
                        nc.tensor.matmul(
                            pm[:],
                            wqk_sb[:, co, fc * P:(fc + 1) * P],
                            xT_sb[:, co, ick * 512:(ick + 1) * 512],
                            start=(co == 0),
                            stop=(co == CO - 1),
                        )
                    nc.vector.tensor_copy(
                        qk_sb[fc][:, ick * 512:(ick + 1) * 512], pm[:]
                    )

            # ---- v in natural layout [n(j), d] + ones column ----
            v_ones = big.tile([P, NB, HEADS_PER_CORE, 65], F32R)
            nc.vector.tensor_copy(
                v_ones[:, :, :, 64:65],
                ones_c.unsqueeze(1).unsqueeze(1).to_broadcast(
                    (P, NB, HEADS_PER_CORE, 1)
                ),
            )
            for nb in range(NB):
                pm = ps_mm.tile([P, 512], F32)
                for co in range(CO):
                    nc.tensor.matmul(
                        pm[:, 0:256],
                        xT_sb[:, co, nb * P:(nb + 1) * P],
                        wv_sb[:, co, :],
                        start=(co == 0),
                        stop=(co == CO - 1),
                    )
                nc.vector.tensor_copy(
                    v_ones[:, nb, :, 0:64],
                    pm[:, 0:256].rearrange("p (h d) -> p h d", h=HEADS_PER_CORE),
                )

            # ---- attention per pair, per i-chunk ----
            # qk_sb index: q of pair p -> 2*p, k of pair p -> 2*p+1
            aoT_sb = [big.tile([P, N], F32R, name=f"aoT_sb{i}") for i in range(2)]
            for pair in range(2):
                q_t = qk_sb[2 * pair]
                k_t = qk_sb[2 * pair + 1]
                hA = 2 * pair
                hB = 2 * pair + 1
                for ick in range(NC4):
                    isl = slice(ick * 512, (ick + 1) * 512)
                    av_A = ps_av.tile([65, 512], F32)
                    av_B = ps_av.tile([65, 512], F32)
                    for jb in range(NB):
                        jsl = slice(jb * P, (jb + 1) * P)
                        sc = ps_sc.tile([P, 2, 512], F32)
                        nc.tensor.matmul(
                            sc[:, 0, :], k_t[0:64, jsl], q_t[0:64, isl],
                            start=True, stop=True,
                        )
                        nc.tensor.matmul(
                            sc[:, 1, :], k_t[64:128, jsl], q_t[64:128, isl],
                            start=True, stop=True,
                        )
                        at = attn_pool.tile([P, 2, 512], F32R)
                        nc.scalar.activation(
                            out=at[:], in_=sc[:],
                            func=mybir.ActivationFunctionType.Exp,
                            scale=float(SCALE),
                        )
                        nc.tensor.matmul(
                            av_A[:], v_ones[:, jb, hA, :], at[:, 0, :],
                            start=(jb == 0), stop=(jb == NB - 1),
                        )
                        nc.tensor.matmul(
                            av_B[:], v_ones[:, jb, hB, :], at[:, 1, :],
                            start=(jb == 0), stop=(jb == NB - 1),
                        )
                        # HAM filler: the PE clock-gate re-throttles to 1.2GHz
                        # unless the PE is near-continuously busy; burn the
                        # ACT-bound slack on junk matmuls (results unused).
                        for jj in range(NUM_FILLER):
                            jp = ps_mm.tile(
                                [64, 256], F32, name="jp", tag="pm"
                            )
                            nc.tensor.matmul(
                                jp[:], warm[:, 0:64], at[:, 1, 0:256],
                                start=True, stop=True,
                            )
                    # Copy av psums to SBUF right away so the PSUM banks free
                    # for the next i-chunk; normalize from SBUF off the
                    # critical path: aoT[d, i] /= sums[i] (row 64 = sums).
                    # Release the av psum banks promptly: copy unnormalized
                    # aoT + sums to SBUF, then normalize aoT in place.
                    sumsA = norm_pool.tile([1, 512], F32)
                    sumsB = norm_pool.tile([1, 512], F32)
                    nc.vector.tensor_copy(aoT_sb[pair][0:64, isl], av_A[0:64, :])
                    nc.vector.tensor_copy(aoT_sb[pair][64:128, isl], av_B[0:64, :])
                    nc.vector.tensor_copy(sumsA[:], av_A[64:65, :])
                    nc.vector.tensor_copy(sumsB[:], av_B[64:65, :])
                    recA = norm_pool.tile([1, 512], F32)
                    recB = norm_pool.tile([1, 512], F32)
                    nc.vector.reciprocal_approx_fast(out=recA[:], in_=sumsA[:])
                    nc.vector.reciprocal_approx_fast(out=recB[:], in_=sumsB[:])
                    rbcA = norm_pool.tile([64, 512], F32)
                    rbcBhi = norm_pool.tile([P, 512], F32)
                    nc.gpsimd.partition_broadcast(rbcA[:], recA[:])
                    nc.gpsimd.partition_broadcast(rbcBhi[0:64, :], recB[:])
                    # DVE SBUF+SBUF inputs must share base partition; shift
                    # head B's recip rows up to partitions 64-127 first.
                    nc.vector.tensor_copy(rbcBhi[64:128, :], rbcBhi[0:64, :])
                    nc.vector.tensor_mul(
                        aoT_sb[pair][0:64, isl], aoT_sb[pair][0:64, isl], rbcA[:]
                    )
                    nc.vector.tensor_mul(
                        aoT_sb[pair][64:128, isl],
                        aoT_sb[pair][64:128, isl],
                        rbcBhi[64:128, :],
                    )

            # ---- output projection (partial over this core's 256 channels) ----
            for nb in range(NB):
                nsl = slice(nb * P, (nb + 1) * P)
                for fck in range(2):
                    fsl = slice(fck * 512, (fck + 1) * 512)
                    pm = ps_mm.tile([P, 512], F32)
                    nc.tensor.matmul(
                        pm[:], aoT_sb[0][:, nsl], wp_sb[:, 0, fsl],
                        start=True, stop=False,
                    )
                    nc.tensor.matmul(
                        pm[:], aoT_sb[1][:, nsl], wp_sb[:, 1, fsl],
                        start=False, stop=True,
                    )
                    ot = out_pool.tile([P, 512], F32)
                    nc.vector.tensor_copy(ot[:], pm[:])
                    nc.sync.dma_start(out.ap()[nsl, fsl], ot[:])

    nc.compile()
    return nc


def _get_nc():
    if "nc" not in _cache:
        _cache["nc"] = _build()
    return _cache["nc"]


def _shard_inputs(x, w_qkv, w_proj):
    """Build per-core input dicts. Core index = b * 4 + g."""
    in_maps = []
    for b in range(B):
        xTb = np.ascontiguousarray(x[b].T)  # [C, N]
        for g in range(4):
            r = g * 256  # head-group row offset within each of q/k/v sections
            wqkT = np.empty((C, 512), np.float32)
            wqkT[:, 0:128] = w_qkv[r:r + 128].T                  # q pair 0
            wqkT[:, 128:256] = w_qkv[C + r:C + r + 128].T        # k pair 0
            wqkT[:, 256:384] = w_qkv[r + 128:r + 256].T          # q pair 1
            wqkT[:, 384:512] = w_qkv[C + r + 128:C + r + 256].T  # k pair 1
            wvT = np.ascontiguousarray(w_qkv[2 * C + r:2 * C + r + 256].T)
            wpT = np.ascontiguousarray(w_proj[:, r:r + 256].T)
            in_maps.append({
                "xT": xTb,
                "wqkT": wqkT,
                "wvT": wvT,
                "wpT": wpT,
            })
    return in_maps


def kernel(x, w_qkv, w_proj, b_proj, _trace=False):
    from concourse.bass_utils import run_bass_kernel_spmd

    x = np.asarray(x, dtype=np.float32)
    w_qkv = np.asarray(w_qkv, dtype=np.float32)
    w_proj = np.asarray(w_proj, dtype=np.float32)
    b_proj = np.asarray(b_proj, dtype=np.float32)

    nc = _get_nc()
    in_maps = _shard_inputs(x, w_qkv, w_proj)
    res = run_bass_kernel_spmd(
        nc, in_maps, core_ids=list(range(N_CORES)), trace=_trace
    )
    out = np.zeros((B, N, C), np.float32)
    for b in range(B):
        for g in range(4):
            out[b] += res.results[b * 4 + g]["out"]
    out += b_proj
    if _trace:
        _cache["last_exec_time_ns"] = res.exec_time_ns
        _cache["last_results"] = res
    return out


# revision 21
# speedup vs baseline: 1.4998x; 1.4998x over previous
"""Multi-head attention (B=2, N=2048, C=1024, H=16, D=64) on 8 Trainium2
NeuronCores.

Sharding: tensor-parallel over heads x data-parallel over batch.
Core (b, g) with b in {0,1}, g in {0..3} handles batch b and heads
[4g, 4g+4). Each core computes qkv for its heads, attention, and a partial
output projection (row-parallel); the host sums the 4 partials per batch and
adds the bias.

Per-core kernel layout (all matmuls in fp32r — full fp32 storage, reduced
multiplier precision, 1 PE cycle/row):
  qT/kT [d, n] via lhsT=w^T, rhs=x^T          (d on partitions, pair-packed)
  scoresT[j, i] = kT.T @ qT                   (two K=64 row-tiled matmuls)
  attnT = exp(scale * scoresT)                (ACT, PSUM->SBUF, no max pass)
  aoT[d, i] += [v | 1]^T @ attnT              (row 64 = softmax denominators)
  aoT *= 1/sums (broadcast), then out = aoT.T @ wpT partial projection.

PSUM pools are phase-scoped (qkv | attention | proj) so the attention phase
gets all 8 banks: scores 2x2 + av 2x2.
"""
import numpy as np
import os
import sys

sys.path.insert(0, "/opt/trn_rl_repo")

B = 2
N = 2048
C = 1024
H = 16
D = 64
SCALE = D ** -0.5

HEADS_PER_CORE = 4  # 2 pairs
N_CORES = 8

_cache = {}


def _build():
    import concourse.bass as bass
    import concourse.tile as tile
    from concourse import bacc, mybir

    F32 = mybir.dt.float32
    F32R = mybir.dt.float32r
    P = 128
    NC4 = N // 512   # 4 i-chunks of 512
    NB = N // P      # 16 n/j blocks of 128
    CO = C // P      # 8 contraction subtiles

    n_warm = int(os.environ.get("K_WARMUP", "24"))

    nc = bacc.Bacc("TRN2", target_bir_lowering=False, debug=False)
    xT = nc.dram_tensor("xT", (C, N), F32, kind="ExternalInput")
    wqkT = nc.dram_tensor("wqkT", (C, 512), F32, kind="ExternalInput")
    wvT = nc.dram_tensor("wvT", (C, 256), F32, kind="ExternalInput")
    wpT = nc.dram_tensor("wpT", (256, C), F32, kind="ExternalInput")
    out = nc.dram_tensor("out", (N, C), F32, kind="ExternalOutput")

    with tile.TileContext(nc) as tc:
        with (
            tc.tile_pool(name="big", bufs=1) as big,
            tc.tile_pool(name="attn", bufs=3) as attn_pool,
            tc.tile_pool(name="norm", bufs=2) as norm_pool,
            tc.tile_pool(name="outp", bufs=3) as out_pool,
        ):
            # ---- weights + x loads; x chunks 0-1 first so qkv starts early
            xT_sb = big.tile([P, CO, N], F32R)
            for co in range(2):
                nc.sync.dma_start(
                    xT_sb[:, co, :],
                    xT.ap()[co * P:(co + 1) * P, :].bitcast(F32R),
                )
            wqk_sb = big.tile([P, CO, 512], F32R)
            for co in range(CO):
                nc.sync.dma_start(
                    wqk_sb[:, co, :],
                    wqkT.ap()[co * P:(co + 1) * P, :].bitcast(F32R),
                )
            for co in range(2, CO):
                nc.sync.dma_start(
                    xT_sb[:, co, :],
                    xT.ap()[co * P:(co + 1) * P, :].bitcast(F32R),
                )
            wv_sb = big.tile([P, CO, 256], F32R)
            for co in range(CO):
                nc.sync.dma_start(
                    wv_sb[:, co, :],
                    wvT.ap()[co * P:(co + 1) * P, :].bitcast(F32R),
                )
            wp_sb = big.tile([P, 2, C], F32R)
            for cs in range(2):
                nc.sync.dma_start(
                    wp_sb[:, cs, :],
                    wpT.ap()[cs * P:(cs + 1) * P, :].bitcast(F32R),
                )
            ones_c = big.tile([P, 1], F32)
            nc.vector.memset(ones_c[:], 1.0)
            # Preload the exp ACT table now (~2.7us) so the first real exp
            # doesn't stall the pipeline at attention start.
            exp_warm = big.tile([P, 1], F32)
            nc.scalar.activation(
                out=exp_warm[:], in_=ones_c[:],
                func=mybir.ActivationFunctionType.Exp,
            )

            warm = big.tile([P, 512], F32R)
            nc.vector.memset(warm[:].bitcast(F32), 0.0)
            wsink = big.tile([P, 8], F32)

            qk_sb = [big.tile([P, N], F32R, name=f"qk_sb{i}") for i in range(4)]
            v_ones = big.tile([P, NB, HEADS_PER_CORE, 65], F32R)
            aoT_sb = [big.tile([P, N], F32R, name=f"aoT_sb{i}") for i in range(2)]

            # ================= phase 1: qkv + v (ps_mm pool) =================
            with tc.tile_pool(name="ps_mm", bufs=2, space="PSUM") as ps_mm:
                # PE warm-up on junk while DMAs stream.
                for wu in range(n_warm):
                    pw = ps_mm.tile([P, 512], F32, name="pwarm", tag="pm")
                    nc.tensor.matmul(
                        pw[:], warm[:, 0:128], warm[:], start=True, stop=True
                    )
                    if wu == n_warm - 1:
                        nc.vector.tensor_copy(wsink[:], pw[:, 0:8])

                # qT / kT (pair-packed [d(2x64), n]);
                # wqkT cols: [q_p0 | k_p0 | q_p1 | k_p1] each 128 wide
                for fc in range(4):
                    for ick in range(NC4):
                        pm = ps_mm.tile([P, 512], F32, name="pm", tag="pm")
                        for co in range(CO):
                            nc.tensor.matmul(
                                pm[:],
                                wqk_sb[:, co, fc * P:(fc + 1) * P],
                                xT_sb[:, co, ick * 512:(ick + 1) * 512],
                                start=(co == 0),
                                stop=(co == CO - 1),
                            )
                        nc.vector.tensor_copy(
                            qk_sb[fc][:, ick * 512:(ick + 1) * 512], pm[:]
                        )

                # v in natural layout [n(j), d] + ones column
                nc.vector.tensor_copy(
                    v_ones[:, :, :, 64:65],
                    ones_c.unsqueeze(1).unsqueeze(1).to_broadcast(
                        (P, NB, HEADS_PER_CORE, 1)
                    ),
                )
                for nb in range(NB):
                    pm = ps_mm.tile([P, 512], F32, name="pm", tag="pm")
                    for co in range(CO):
                        nc.tensor.matmul(
                            pm[:, 0:256],
                            xT_sb[:, co, nb * P:(nb + 1) * P],
                            wv_sb[:, co, :],
                            start=(co == 0),
                            stop=(co == CO - 1),
                        )
                    nc.vector.tensor_copy(
                        v_ones[:, nb, :, 0:64],
                        pm[:, 0:256].rearrange(
                            "p (h d) -> p h d", h=HEADS_PER_CORE
                        ),
                    )

            # ================= phase 2: attention ===========================
            with (
                tc.tile_pool(name="ps_sc", bufs=2, space="PSUM") as ps_sc,
                tc.tile_pool(name="ps_av", bufs=2, space="PSUM") as ps_av,
            ):
                for pair in range(2):
                    q_t = qk_sb[2 * pair]
                    k_t = qk_sb[2 * pair + 1]
                    hA = 2 * pair
                    hB = 2 * pair + 1
                    for ick in range(NC4):
                        isl = slice(ick * 512, (ick + 1) * 512)
                        av_A = ps_av.tile([65, 512], F32)
                        av_B = ps_av.tile([65, 512], F32)
                        for jb in range(NB):
                            jsl = slice(jb * P, (jb + 1) * P)
                            sc = ps_sc.tile([P, 2, 512], F32)
                            nc.tensor.matmul(
                                sc[:, 0, :], k_t[0:64, jsl], q_t[0:64, isl],
                                start=True, stop=True,
                            )
                            nc.tensor.matmul(
                                sc[:, 1, :], k_t[64:128, jsl], q_t[64:128, isl],
                                start=True, stop=True,
                            )
                            at = attn_pool.tile([P, 2, 512], F32R)
                            nc.scalar.activation(
                                out=at[:], in_=sc[:],
                                func=mybir.ActivationFunctionType.Exp,
                                scale=float(SCALE),
                            )
                            nc.tensor.matmul(
                                av_A[:], v_ones[:, jb, hA, :], at[:, 0, :],
                                start=(jb == 0), stop=(jb == NB - 1),
                            )
                            nc.tensor.matmul(
                                av_B[:], v_ones[:, jb, hB, :], at[:, 1, :],
                                start=(jb == 0), stop=(jb == NB - 1),
                            )
                        # Release the av psum banks promptly: copy unnormalized
                        # aoT + sums to SBUF, then normalize aoT in place.
                        sumsA = norm_pool.tile([1, 512], F32)
                        sumsB = norm_pool.tile([1, 512], F32)
                        nc.vector.tensor_copy(
                            aoT_sb[pair][0:64, isl], av_A[0:64, :]
                        )
                        nc.vector.tensor_copy(
                            aoT_sb[pair][64:128, isl], av_B[0:64, :]
                        )
                        nc.vector.tensor_copy(sumsA[:], av_A[64:65, :])
                        nc.vector.tensor_copy(sumsB[:], av_B[64:65, :])
                        recA = norm_pool.tile([1, 512], F32)
                        recB = norm_pool.tile([1, 512], F32)
                        nc.vector.reciprocal_approx_fast(
                            out=recA[:], in_=sumsA[:]
                        )
                        nc.vector.reciprocal_approx_fast(
                            out=recB[:], in_=sumsB[:]
                        )
                        rbcA = norm_pool.tile([64, 512], F32)
                        rbcBhi = norm_pool.tile([P, 512], F32)
                        nc.gpsimd.partition_broadcast(rbcA[:], recA[:])
                        nc.gpsimd.partition_broadcast(rbcBhi[0:64, :], recB[:])
                        # DVE SBUF+SBUF inputs must share base partition; shift
                        # head B's recip rows up to partitions 64-127 first.
                        nc.vector.tensor_copy(
                            rbcBhi[64:128, :], rbcBhi[0:64, :]
                        )
                        nc.vector.tensor_mul(
                            aoT_sb[pair][0:64, isl],
                            aoT_sb[pair][0:64, isl],
                            rbcA[:],
                        )
                        nc.vector.tensor_mul(
                            aoT_sb[pair][64:128, isl],
                            aoT_sb[pair][64:128, isl],
                            rbcBhi[64:128, :],
                        )

            # ================= phase 3: output projection ===================
            with tc.tile_pool(name="ps_pj", bufs=2, space="PSUM") as ps_pj:
                for nb in range(NB):
                    nsl = slice(nb * P, (nb + 1) * P)
                    for fck in range(2):
                        fsl = slice(fck * 512, (fck + 1) * 512)
                        pj = ps_pj.tile([P, 512], F32)
                        nc.tensor.matmul(
                            pj[:], aoT_sb[0][:, nsl], wp_sb[:, 0, fsl],
                            start=True, stop=False,
                        )
                        nc.tensor.matmul(
                            pj[:], aoT_sb[1][:, nsl], wp_sb[:, 1, fsl],
                            start=False, stop=True,
                        )
                        ot = out_pool.tile([P, 512], F32)
                        nc.vector.tensor_copy(ot[:], pj[:])
                        nc.sync.dma_start(out.ap()[nsl, fsl], ot[:])

    nc.compile()
    return nc


def _get_nc():
    if "nc" not in _cache:
        _cache["nc"] = _build()
    return _cache["nc"]


def _shard_inputs(x, w_qkv, w_proj):
    """Build per-core input dicts. Core index = b * 4 + g."""
    in_maps = []
    for b in range(B):
        xTb = np.ascontiguousarray(x[b].T)  # [C, N]
        for g in range(4):
            r = g * 256  # head-group row offset within each of q/k/v sections
            wqkT = np.empty((C, 512), np.float32)
            wqkT[:, 0:128] = w_qkv[r:r + 128].T                  # q pair 0
            wqkT[:, 128:256] = w_qkv[C + r:C + r + 128].T        # k pair 0
            wqkT[:, 256:384] = w_qkv[r + 128:r + 256].T          # q pair 1
            wqkT[:, 384:512] = w_qkv[C + r + 128:C + r + 256].T  # k pair 1
            wvT = np.ascontiguousarray(w_qkv[2 * C + r:2 * C + r + 256].T)
            wpT = np.ascontiguousarray(w_proj[:, r:r + 256].T)
            in_maps.append({
                "xT": xTb,
                "wqkT": wqkT,
                "wvT": wvT,
                "wpT": wpT,
            })
    return in_maps


def kernel(x, w_qkv, w_proj, b_proj, _trace=False):
    from concourse.bass_utils import run_bass_kernel_spmd

    x = np.asarray(x, dtype=np.float32)
    w_qkv = np.asarray(w_qkv, dtype=np.float32)
    w_proj = np.asarray(w_proj, dtype=np.float32)
    b_proj = np.asarray(b_proj, dtype=np.float32)

    nc = _get_nc()
    in_maps = _shard_inputs(x, w_qkv, w_proj)
    res = run_bass_kernel_spmd(
        nc, in_maps, core_ids=list(range(N_CORES)), trace=_trace
    )
    out = np.zeros((B, N, C), np.float32)
    for b in range(B):
        for g in range(4):
            out[b] += res.results[b * 4 + g]["out"]
    out += b_proj
    if _trace:
        _cache["last_exec_time_ns"] = res.exec_time_ns
        _cache["last_results"] = res
    return out


# revision 22
# speedup vs baseline: 1.5692x; 1.0463x over previous
"""Multi-head attention (B=2, N=2048, C=1024, H=16, D=64) on 8 Trainium2
NeuronCores.

Sharding: tensor-parallel over heads x data-parallel over batch.
Core (b, g) with b in {0,1}, g in {0..3} handles batch b and heads
[4g, 4g+4). Each core computes qkv for its heads, attention, and a partial
output projection (row-parallel); the host sums the 4 partials per batch and
adds the bias.

Per-core kernel (all matmuls fp32r — fp32 storage, reduced-precision multiply,
1 PE cycle/row):
  qT/kT [d, n] via lhsT=w^T, rhs=x^T          (d on partitions, pair-packed)
  scoresT[j, i] = kT.T @ qT                   (two K=64 row-tiled matmuls)
  attnT = exp(scale * scoresT)                (ACT, PSUM->SBUF, no max pass)
  aoT[d, i] += [v | 1]^T @ attnT              (row 64 = softmax denominators)
  aoT *= 1/sums (broadcast), then out = aoT.T @ wpT partial projection.

The ACT engine (exp) is the saturated resource (~140us). Emission order is
arranged so all non-attention PE work (qkv for pair 1, the output projection)
fills the PE idle slots inside the ACT-bound attention phase:
  [dma x,w | warmup] k_p0,q_p0,v -> attn p0 | qkv p1 -> attn p1 | proj(chunk)
"""
import numpy as np
import os
import sys

sys.path.insert(0, "/opt/trn_rl_repo")

B = 2
N = 2048
C = 1024
H = 16
D = 64
SCALE = D ** -0.5

HEADS_PER_CORE = 4  # 2 pairs
N_CORES = 8

_cache = {}


def _build():
    import concourse.bass as bass
    import concourse.tile as tile
    from concourse import bacc, mybir

    F32 = mybir.dt.float32
    F32R = mybir.dt.float32r
    P = 128
    NC4 = N // 512   # 4 i-chunks of 512
    NB = N // P      # 16 n/j blocks of 128
    CO = C // P      # 8 contraction subtiles

    n_warm = int(os.environ.get("K_WARMUP", "24"))

    nc = bacc.Bacc("TRN2", target_bir_lowering=False, debug=False)
    xT = nc.dram_tensor("xT", (C, N), F32, kind="ExternalInput")
    wqkT = nc.dram_tensor("wqkT", (C, 512), F32, kind="ExternalInput")
    wvT = nc.dram_tensor("wvT", (C, 256), F32, kind="ExternalInput")
    wpT = nc.dram_tensor("wpT", (256, C), F32, kind="ExternalInput")
    out = nc.dram_tensor("out", (N, C), F32, kind="ExternalOutput")

    with tile.TileContext(nc) as tc:
        with (
            tc.tile_pool(name="big", bufs=1) as big,
            tc.tile_pool(name="attn", bufs=3) as attn_pool,
            tc.tile_pool(name="norm", bufs=2) as norm_pool,
            tc.tile_pool(name="outp", bufs=3) as out_pool,
            tc.tile_pool(name="ps_mm", bufs=2, space="PSUM") as ps_mm,
            tc.tile_pool(name="ps_sc", bufs=2, space="PSUM") as ps_sc,
            tc.tile_pool(name="ps_av", bufs=1, space="PSUM") as ps_av,
        ):
            # ---- input DMAs: xT first (largest, gates everything) ----
            xT_sb = big.tile([P, CO, N], F32R)
            for co in range(CO):
                nc.sync.dma_start(
                    xT_sb[:, co, :],
                    xT.ap()[co * P:(co + 1) * P, :].bitcast(F32R),
                )
            wqk_sb = big.tile([P, CO, 512], F32R)
            for co in range(CO):
                nc.sync.dma_start(
                    wqk_sb[:, co, :],
                    wqkT.ap()[co * P:(co + 1) * P, :].bitcast(F32R),
                )
            wv_sb = big.tile([P, CO, 256], F32R)
            for co in range(CO):
                nc.sync.dma_start(
                    wv_sb[:, co, :],
                    wvT.ap()[co * P:(co + 1) * P, :].bitcast(F32R),
                )
            wp_sb = big.tile([P, 2, C], F32R)
            for cs in range(2):
                nc.sync.dma_start(
                    wp_sb[:, cs, :],
                    wpT.ap()[cs * P:(cs + 1) * P, :].bitcast(F32R),
                )
            ones_c = big.tile([P, 1], F32)
            nc.vector.memset(ones_c[:], 1.0)
            # Preload the exp ACT table (~2.7us) during the DMA lead-in.
            exp_warm = big.tile([P, 1], F32)
            nc.scalar.activation(
                out=exp_warm[:], in_=ones_c[:],
                func=mybir.ActivationFunctionType.Exp,
            )

            warm = big.tile([P, 512], F32R)
            nc.vector.memset(warm[:].bitcast(F32), 0.0)
            wsink = big.tile([P, 8], F32)
            for wu in range(n_warm):
                pw = ps_mm.tile([P, 512], F32, name="pwarm", tag="pm")
                nc.tensor.matmul(
                    pw[:], warm[:, 0:128], warm[:], start=True, stop=True
                )
                if wu == n_warm - 1:
                    nc.vector.tensor_copy(wsink[:], pw[:, 0:8])

            qk_sb = [big.tile([P, N], F32R, name=f"qk_sb{i}") for i in range(4)]
            # per-j-block v tiles (fine-grained deps so attention j=0 does not
            # wait for the whole v phase)
            v_ones = [
                big.tile([P, HEADS_PER_CORE, 65], F32R, name=f"vo{nb}")
                for nb in range(NB)
            ]
            aoT_sb = [big.tile([P, N], F32R, name=f"aoT_sb{i}") for i in range(2)]

            def qk_chains(fc):
                """qT or kT f-chunk fc -> qk_sb[fc] [d(2x64 pair-packed), n]"""
                for ick in range(NC4):
                    pm = ps_mm.tile([P, 512], F32, name="pm", tag="pm")
                    for co in range(CO):
                        nc.tensor.matmul(
                            pm[:],
                            wqk_sb[:, co, fc * P:(fc + 1) * P],
                            xT_sb[:, co, ick * 512:(ick + 1) * 512],
                            start=(co == 0),
                            stop=(co == CO - 1),
                        )
                    nc.vector.tensor_copy(
                        qk_sb[fc][:, ick * 512:(ick + 1) * 512], pm[:]
                    )

            def v_chain(nb):
                pm = ps_mm.tile([P, 512], F32, name="pm", tag="pm")
                for co in range(CO):
                    nc.tensor.matmul(
                        pm[:, 0:256],
                        xT_sb[:, co, nb * P:(nb + 1) * P],
                        wv_sb[:, co, :],
                        start=(co == 0),
                        stop=(co == CO - 1),
                    )
                nc.vector.tensor_copy(
                    v_ones[nb][:, :, 0:64],
                    pm[:, 0:256].rearrange("p (h d) -> p h d", h=HEADS_PER_CORE),
                )
                nc.vector.tensor_copy(
                    v_ones[nb][:, :, 64:65],
                    ones_c.unsqueeze(1).to_broadcast((P, HEADS_PER_CORE, 1)),
                )

            def attention_chunk(pair, ick):
                q_t = qk_sb[2 * pair]
                k_t = qk_sb[2 * pair + 1]
                hA = 2 * pair
                hB = 2 * pair + 1
                isl = slice(ick * 512, (ick + 1) * 512)
                av_A = ps_av.tile([65, 512], F32, name="av_A")
                av_B = ps_av.tile([65, 512], F32, name="av_B")
                for jb in range(NB):
                    jsl = slice(jb * P, (jb + 1) * P)
                    sc = ps_sc.tile([P, 2, 512], F32, name="sc")
                    nc.tensor.matmul(
                        sc[:, 0, :], k_t[0:64, jsl], q_t[0:64, isl],
                        start=True, stop=True,
                    )
                    nc.tensor.matmul(
                        sc[:, 1, :], k_t[64:128, jsl], q_t[64:128, isl],
                        start=True, stop=True,
                    )
                    at = attn_pool.tile([P, 2, 512], F32R, name="at")
                    nc.scalar.activation(
                        out=at[:], in_=sc[:],
                        func=mybir.ActivationFunctionType.Exp,
                        scale=float(SCALE),
                    )
                    nc.tensor.matmul(
                        av_A[:], v_ones[jb][:, hA, :], at[:, 0, :],
                        start=(jb == 0), stop=(jb == NB - 1),
                    )
                    nc.tensor.matmul(
                        av_B[:], v_ones[jb][:, hB, :], at[:, 1, :],
                        start=(jb == 0), stop=(jb == NB - 1),
                    )
                # Copy unnormalized aoT + sums to SBUF (frees the av banks),
                # then normalize aoT in place.
                sumsA = norm_pool.tile([1, 512], F32, name="sumsA")
                sumsB = norm_pool.tile([1, 512], F32, name="sumsB")
                nc.vector.tensor_copy(aoT_sb[pair][0:64, isl], av_A[0:64, :])
                nc.vector.tensor_copy(aoT_sb[pair][64:128, isl], av_B[0:64, :])
                nc.vector.tensor_copy(sumsA[:], av_A[64:65, :])
                nc.vector.tensor_copy(sumsB[:], av_B[64:65, :])
                recA = norm_pool.tile([1, 512], F32, name="recA")
                recB = norm_pool.tile([1, 512], F32, name="recB")
                nc.vector.reciprocal_approx_fast(out=recA[:], in_=sumsA[:])
                nc.vector.reciprocal_approx_fast(out=recB[:], in_=sumsB[:])
                rbcA = norm_pool.tile([64, 512], F32, name="rbcA")
                rbcBhi = norm_pool.tile([P, 512], F32, name="rbcBhi")
                nc.gpsimd.partition_broadcast(rbcA[:], recA[:])
                nc.gpsimd.partition_broadcast(rbcBhi[0:64, :], recB[:])
                # DVE SBUF+SBUF inputs must share base partition; shift head
                # B's recip rows up to partitions 64-127 first.
                nc.vector.tensor_copy(rbcBhi[64:128, :], rbcBhi[0:64, :])
                nc.vector.tensor_mul(
                    aoT_sb[pair][0:64, isl], aoT_sb[pair][0:64, isl], rbcA[:]
                )
                nc.vector.tensor_mul(
                    aoT_sb[pair][64:128, isl],
                    aoT_sb[pair][64:128, isl],
                    rbcBhi[64:128, :],
                )

            def proj_block(nb):
                nsl = slice(nb * P, (nb + 1) * P)
                for fck in range(2):
                    fsl = slice(fck * 512, (fck + 1) * 512)
                    pj = ps_mm.tile([P, 512], F32, name="pj", tag="pm")
                    nc.tensor.matmul(
                        pj[:], aoT_sb[0][:, nsl], wp_sb[:, 0, fsl],
                        start=True, stop=False,
                    )
                    nc.tensor.matmul(
                        pj[:], aoT_sb[1][:, nsl], wp_sb[:, 1, fsl],
                        start=False, stop=True,
                    )
                    ot = out_pool.tile([P, 512], F32, name="ot")
                    nc.vector.tensor_copy(ot[:], pj[:])
                    nc.sync.dma_start(out.ap()[nsl, fsl], ot[:])

            # ---- emission: attention p0 first; pair-1 qkv and the
            # projection are emitted inside/after so the scheduler slots
            # their PE work into the ACT-bound attention idles.
            qk_chains(1)          # k_p0
            qk_chains(0)          # q_p0
            for nb in range(NB):
                v_chain(nb)
            for ick in range(NC4):
                attention_chunk(0, ick)
                if ick == 0:
                    qk_chains(3)  # k_p1
                elif ick == 1:
                    qk_chains(2)  # q_p1
            for ick in range(NC4):
                attention_chunk(1, ick)
                # proj for the i-rows whose pair-1 aoT chunk just completed
                for nb in range(4 * ick, 4 * ick + 4):
                    proj_block(nb)

    nc.compile()
    return nc


def _get_nc():
    if "nc" not in _cache:
        _cache["nc"] = _build()
    return _cache["nc"]


def _shard_inputs(x, w_qkv, w_proj):
    """Build per-core input dicts. Core index = b * 4 + g."""
    in_maps = []
    for b in range(B):
        xTb = np.ascontiguousarray(x[b].T)  # [C, N]
        for g in range(4):
            r = g * 256  # head-group row offset within each of q/k/v sections
            wqkT = np.empty((C, 512), np.float32)
            wqkT[:, 0:128] = w_qkv[r:r + 128].T                  # q pair 0
            wqkT[:, 128:256] = w_qkv[C + r:C + r + 128].T        # k pair 0
            wqkT[:, 256:384] = w_qkv[r + 128:r + 256].T          # q pair 1
            wqkT[:, 384:512] = w_qkv[C + r + 128:C + r + 256].T  # k pair 1
            wvT = np.ascontiguousarray(w_qkv[2 * C + r:2 * C + r + 256].T)
            wpT = np.ascontiguousarray(w_proj[:, r:r + 256].T)
            in_maps.append({
                "xT": xTb,
                "wqkT": wqkT,
                "wvT": wvT,
                "wpT": wpT,
            })
    return in_maps


def kernel(x, w_qkv, w_proj, b_proj, _trace=False):
    from concourse.bass_utils import run_bass_kernel_spmd

    x = np.asarray(x, dtype=np.float32)
    w_qkv = np.asarray(w_qkv, dtype=np.float32)
    w_proj = np.asarray(w_proj, dtype=np.float32)
    b_proj = np.asarray(b_proj, dtype=np.float32)

    nc = _get_nc()
    in_maps = _shard_inputs(x, w_qkv, w_proj)
    res = run_bass_kernel_spmd(
        nc, in_maps, core_ids=list(range(N_CORES)), trace=_trace
    )
    out = np.zeros((B, N, C), np.float32)
    for b in range(B):
        for g in range(4):
            out[b] += res.results[b * 4 + g]["out"]
    out += b_proj
    if _trace:
        _cache["last_exec_time_ns"] = res.exec_time_ns
        _cache["last_results"] = res
    return out


# revision 25
# speedup vs baseline: 1.6078x; 1.0246x over previous
"""Multi-head attention (B=2, N=2048, C=1024, H=16, D=64) on 8 Trainium2
NeuronCores.

Sharding: tensor-parallel over heads x data-parallel over batch.
Core (b, g) with b in {0,1}, g in {0..3} handles batch b and heads
[4g, 4g+4). Each core computes qkv for its heads, attention, and a partial
output projection (row-parallel); the host sums the 4 partials per batch and
adds the bias.

Per-core kernel (all matmuls fp32r — fp32 storage, reduced-precision multiply,
1 PE cycle/row):
  qT/kT [d, n] via lhsT=w^T, rhs=x^T          (d on partitions, pair-packed)
  scoresT[j, i] = kT.T @ qT                   (two K=64 row-tiled matmuls)
  attnT = exp(scale * scoresT)                (ACT, PSUM->SBUF, no max pass)
  aoT[d, i] += [v | 1]^T @ attnT              (row 64 = softmax denominators)
  aoT *= 1/sums (broadcast), then out = aoT.T @ wpT partial projection.

The ACT engine (exp) is the saturated resource (~140us). Emission order is
arranged so all non-attention PE work (qkv for pair 1, the output projection)
fills the PE idle slots inside the ACT-bound attention phase:
  [dma x,w | warmup] k_p0,q_p0,v -> attn p0 | qkv p1 -> attn p1 | proj(chunk)
"""
import numpy as np
import os
import sys

sys.path.insert(0, "/opt/trn_rl_repo")

B = 2
N = 2048
C = 1024
H = 16
D = 64
SCALE = D ** -0.5

HEADS_PER_CORE = 4  # 2 pairs
N_CORES = 8

_cache = {}


def _build():
    import concourse.bass as bass
    import concourse.tile as tile
    from concourse import bacc, mybir

    F32 = mybir.dt.float32
    F32R = mybir.dt.float32r
    P = 128
    NC4 = N // 512   # 4 i-chunks of 512
    NB = N // P      # 16 n/j blocks of 128
    CO = C // P      # 8 contraction subtiles

    n_warm = int(os.environ.get("K_WARMUP", "24"))

    nc = bacc.Bacc("TRN2", target_bir_lowering=False, debug=False)
    xT = nc.dram_tensor("xT", (C, N), F32, kind="ExternalInput")
    wqkT = nc.dram_tensor("wqkT", (C, 512), F32, kind="ExternalInput")
    wvT = nc.dram_tensor("wvT", (C, 256), F32, kind="ExternalInput")
    wpT = nc.dram_tensor("wpT", (256, C), F32, kind="ExternalInput")
    out = nc.dram_tensor("out", (N, C), F32, kind="ExternalOutput")

    with tile.TileContext(nc) as tc:
        with (
            tc.tile_pool(name="big", bufs=1) as big,
            tc.tile_pool(name="attn", bufs=3) as attn_pool,
            tc.tile_pool(name="norm", bufs=2) as norm_pool,
            tc.tile_pool(name="outp", bufs=3) as out_pool,
            tc.tile_pool(name="ps_mm", bufs=2, space="PSUM") as ps_mm,
            tc.tile_pool(name="ps_sc", bufs=2, space="PSUM") as ps_sc,
            tc.tile_pool(name="ps_av", bufs=1, space="PSUM") as ps_av,
        ):
            # ---- input DMAs: xT first (largest, gates everything) ----
            xT_sb = big.tile([P, CO, N], F32R)
            for co in range(CO):
                nc.sync.dma_start(
                    xT_sb[:, co, :],
                    xT.ap()[co * P:(co + 1) * P, :].bitcast(F32R),
                )
            wqk_sb = big.tile([P, CO, 512], F32R)
            for co in range(CO):
                nc.sync.dma_start(
                    wqk_sb[:, co, :],
                    wqkT.ap()[co * P:(co + 1) * P, :].bitcast(F32R),
                )
            wv_sb = big.tile([P, CO, 256], F32R)
            for co in range(CO):
                nc.sync.dma_start(
                    wv_sb[:, co, :],
                    wvT.ap()[co * P:(co + 1) * P, :].bitcast(F32R),
                )
            wp_sb = big.tile([P, 2, C], F32R)
            for cs in range(2):
                nc.sync.dma_start(
                    wp_sb[:, cs, :],
                    wpT.ap()[cs * P:(cs + 1) * P, :].bitcast(F32R),
                )
            ones_c = big.tile([P, 1], F32)
            nc.vector.memset(ones_c[:], 1.0)
            # Preload the exp ACT table (~2.7us) during the DMA lead-in.
            exp_warm = big.tile([P, 1], F32)
            nc.scalar.activation(
                out=exp_warm[:], in_=ones_c[:],
                func=mybir.ActivationFunctionType.Exp,
            )

            warm = big.tile([P, 512], F32R)
            nc.vector.memset(warm[:].bitcast(F32), 0.0)
            wsink = big.tile([P, 8], F32)
            for wu in range(n_warm):
                pw = ps_mm.tile([P, 512], F32, name="pwarm", tag="pm")
                nc.tensor.matmul(
                    pw[:], warm[:, 0:128], warm[:], start=True, stop=True
                )
                if wu == n_warm - 1:
                    nc.vector.tensor_copy(wsink[:], pw[:, 0:8])

            qk_sb = [big.tile([P, N], F32R, name=f"qk_sb{i}") for i in range(4)]
            # per-j-block v tiles (fine-grained deps so attention j=0 does not
            # wait for the whole v phase)
            v_ones = [
                big.tile([P, HEADS_PER_CORE, 65], F32R, name=f"vo{nb}")
                for nb in range(NB)
            ]
            aoT_sb = [big.tile([P, N], F32R, name=f"aoT_sb{i}") for i in range(2)]

            def qk_chain(fc, ick):
                """qT or kT f-chunk fc, n-chunk ick -> qk_sb[fc]"""
                pm = ps_mm.tile([P, 512], F32, name="pm", tag="pm")
                for co in range(CO):
                    nc.tensor.matmul(
                        pm[:],
                        wqk_sb[:, co, fc * P:(fc + 1) * P],
                        xT_sb[:, co, ick * 512:(ick + 1) * 512],
                        start=(co == 0),
                        stop=(co == CO - 1),
                    )
                nc.vector.tensor_copy(
                    qk_sb[fc][:, ick * 512:(ick + 1) * 512], pm[:]
                )

            def qk_chains(fc):
                for ick in range(NC4):
                    qk_chain(fc, ick)

            def v_chain(nb):
                pm = ps_mm.tile([P, 512], F32, name="pm", tag="pm")
                for co in range(CO):
                    nc.tensor.matmul(
                        pm[:, 0:256],
                        xT_sb[:, co, nb * P:(nb + 1) * P],
                        wv_sb[:, co, :],
                        start=(co == 0),
                        stop=(co == CO - 1),
                    )
                nc.vector.tensor_copy(
                    v_ones[nb][:, :, 0:64],
                    pm[:, 0:256].rearrange("p (h d) -> p h d", h=HEADS_PER_CORE),
                )
                nc.vector.tensor_copy(
                    v_ones[nb][:, :, 64:65],
                    ones_c.unsqueeze(1).to_broadcast((P, HEADS_PER_CORE, 1)),
                )

            def attention_chunk(pair, ick, filler=None):
                q_t = qk_sb[2 * pair]
                k_t = qk_sb[2 * pair + 1]
                hA = 2 * pair
                hB = 2 * pair + 1
                isl = slice(ick * 512, (ick + 1) * 512)
                av_A = ps_av.tile([65, 512], F32, name="av_A")
                av_B = ps_av.tile([65, 512], F32, name="av_B")
                for jb in range(NB):
                    jsl = slice(jb * P, (jb + 1) * P)
                    sc = ps_sc.tile([P, 2, 512], F32, name="sc")
                    nc.tensor.matmul(
                        sc[:, 0, :], k_t[0:64, jsl], q_t[0:64, isl],
                        start=True, stop=True,
                    )
                    nc.tensor.matmul(
                        sc[:, 1, :], k_t[64:128, jsl], q_t[64:128, isl],
                        start=True, stop=True,
                    )
                    at = attn_pool.tile([P, 2, 512], F32R, name="at")
                    nc.scalar.activation(
                        out=at[:], in_=sc[:],
                        func=mybir.ActivationFunctionType.Exp,
                        scale=float(SCALE),
                    )
                    nc.tensor.matmul(
                        av_A[:], v_ones[jb][:, hA, :], at[:, 0, :],
                        start=(jb == 0), stop=(jb == NB - 1),
                    )
                    nc.tensor.matmul(
                        av_B[:], v_ones[jb][:, hB, :], at[:, 1, :],
                        start=(jb == 0), stop=(jb == NB - 1),
                    )
                    # low-priority filler (emitted after the latency-critical
                    # attention ops of this iteration)
                    if filler is not None and jb in filler:
                        filler[jb]()
                # Copy unnormalized aoT + sums to SBUF (frees the av banks),
                # then normalize aoT in place.
                sumsA = norm_pool.tile([1, 512], F32, name="sumsA")
                sumsB = norm_pool.tile([1, 512], F32, name="sumsB")
                nc.vector.tensor_copy(aoT_sb[pair][0:64, isl], av_A[0:64, :])
                nc.vector.tensor_copy(aoT_sb[pair][64:128, isl], av_B[0:64, :])
                nc.vector.tensor_copy(sumsA[:], av_A[64:65, :])
                nc.vector.tensor_copy(sumsB[:], av_B[64:65, :])
                recA = norm_pool.tile([1, 512], F32, name="recA")
                recB = norm_pool.tile([1, 512], F32, name="recB")
                nc.vector.reciprocal_approx_fast(out=recA[:], in_=sumsA[:])
                nc.vector.reciprocal_approx_fast(out=recB[:], in_=sumsB[:])
                rbcA = norm_pool.tile([64, 512], F32, name="rbcA")
                rbcBhi = norm_pool.tile([P, 512], F32, name="rbcBhi")
                nc.gpsimd.partition_broadcast(rbcA[:], recA[:])
                nc.gpsimd.partition_broadcast(rbcBhi[0:64, :], recB[:])
                # DVE SBUF+SBUF inputs must share base partition; shift head
                # B's recip rows up to partitions 64-127 first.
                nc.vector.tensor_copy(rbcBhi[64:128, :], rbcBhi[0:64, :])
                nc.vector.tensor_mul(
                    aoT_sb[pair][0:64, isl], aoT_sb[pair][0:64, isl], rbcA[:]
                )
                nc.vector.tensor_mul(
                    aoT_sb[pair][64:128, isl],
                    aoT_sb[pair][64:128, isl],
                    rbcBhi[64:128, :],
                )

            def proj_block(nb):
                nsl = slice(nb * P, (nb + 1) * P)
                for fck in range(2):
                    fsl = slice(fck * 512, (fck + 1) * 512)
                    pj = ps_mm.tile([P, 512], F32, name="pj", tag="pm")
                    nc.tensor.matmul(
                        pj[:], aoT_sb[0][:, nsl], wp_sb[:, 0, fsl],
                        start=True, stop=False,
                    )
                    nc.tensor.matmul(
                        pj[:], aoT_sb[1][:, nsl], wp_sb[:, 1, fsl],
                        start=False, stop=True,
                    )
                    ot = out_pool.tile([P, 512], F32, name="ot")
                    nc.vector.tensor_copy(ot[:], pj[:])
                    nc.sync.dma_start(out.ap()[nsl, fsl], ot[:])

            # ---- emission: attention p0 as early as possible; v chains,
            # pair-1 qkv, and the projection are interleaved into the
            # ACT-bound attention iterations as low-priority PE filler.
            qk_chains(1)          # k_p0 (full n, gates all scores)
            qk_chains(0)          # q_p0
            v_chain(0)
            v_chain(1)
            # p0 chunk 0: emit remaining v chains two iterations ahead of use
            attention_chunk(
                0, 0,
                filler={jb: (lambda nb=jb + 2: v_chain(nb))
                        for jb in range(NB - 2)},
            )
            # p0 chunks 1-2: slot in pair-1 k/q chains (one chain per 4 iters)
            attention_chunk(
                0, 1,
                filler={4 * i + 3: (lambda i=i: qk_chain(3, i)) for i in range(4)},
            )
            attention_chunk(
                0, 2,
                filler={4 * i + 3: (lambda i=i: qk_chain(2, i)) for i in range(4)},
            )
            attention_chunk(0, 3)
            # pair 1; proj for chunk c interleaves into chunk c+1
            for ick in range(NC4):
                fill = {}
                if ick > 0:
                    base = 4 * (ick - 1)
                    fill = {4 * i + 3: (lambda nb=base + i: proj_block(nb))
                            for i in range(4)}
                attention_chunk(1, ick, filler=fill)
            for nb in range(12, 16):
                proj_block(nb)

    nc.compile()
    return nc


def _get_nc():
    if "nc" not in _cache:
        _cache["nc"] = _build()
    return _cache["nc"]


def _shard_inputs(x, w_qkv, w_proj):
    """Build per-core input dicts. Core index = b * 4 + g."""
    in_maps = []
    for b in range(B):
        xTb = np.ascontiguousarray(x[b].T)  # [C, N]
        for g in range(4):
            r = g * 256  # head-group row offset within each of q/k/v sections
            wqkT = np.empty((C, 512), np.float32)
            wqkT[:, 0:128] = w_qkv[r:r + 128].T                  # q pair 0
            wqkT[:, 128:256] = w_qkv[C + r:C + r + 128].T        # k pair 0
            wqkT[:, 256:384] = w_qkv[r + 128:r + 256].T          # q pair 1
            wqkT[:, 384:512] = w_qkv[C + r + 128:C + r + 256].T  # k pair 1
            wvT = np.ascontiguousarray(w_qkv[2 * C + r:2 * C + r + 256].T)
            wpT = np.ascontiguousarray(w_proj[:, r:r + 256].T)
            in_maps.append({
                "xT": xTb,
                "wqkT": wqkT,
                "wvT": wvT,
                "wpT": wpT,
            })
    return in_maps


def kernel(x, w_qkv, w_proj, b_proj, _trace=False):
    from concourse.bass_utils import run_bass_kernel_spmd

    x = np.asarray(x, dtype=np.float32)
    w_qkv = np.asarray(w_qkv, dtype=np.float32)
    w_proj = np.asarray(w_proj, dtype=np.float32)
    b_proj = np.asarray(b_proj, dtype=np.float32)

    nc = _get_nc()
    in_maps = _shard_inputs(x, w_qkv, w_proj)
    res = run_bass_kernel_spmd(
        nc, in_maps, core_ids=list(range(N_CORES)), trace=_trace
    )
    out = np.zeros((B, N, C), np.float32)
    for b in range(B):
        for g in range(4):
            out[b] += res.results[b * 4 + g]["out"]
    out += b_proj
    if _trace:
        _cache["last_exec_time_ns"] = res.exec_time_ns
        _cache["last_results"] = res
    return out
